# revision 11
# baseline (speedup 1.0000x reference)
import os
import numpy as np
import concourse.bass as bass
from concourse import bacc
import concourse.tile as tile
from concourse import mybir
from concourse.bass_utils import run_bass_kernel_spmd

WINDOW = 8
HEADS = 12
HD = 64
DISP = 4
WS2 = 64
NWIN = 64
B = 8
HP = 6  # head pairs


def _rel_indices():
    idx = np.array([[x, y] for x in range(WINDOW) for y in range(WINDOW)])
    rel = idx[None, :, :] - idx[:, None, :] + (WINDOW - 1)
    return rel


def _create_mask(upper_lower, left_right):
    m = np.zeros((WS2, WS2), dtype=np.float32)
    d = DISP
    if upper_lower:
        m[-d * WINDOW:, :-d * WINDOW] = -np.inf
        m[:-d * WINDOW, -d * WINDOW:] = -np.inf
    if left_right:
        m4 = m.reshape(WINDOW, WINDOW, WINDOW, WINDOW)
        m4[:, -d:, :, :-d] = -np.inf
        m4[:, :-d, :, -d:] = -np.inf
        m = m4.reshape(WS2, WS2)
    return m


def _window_masks(n_h, n_w):
    ul = _create_mask(True, False)
    lr = _create_mask(False, True)
    masks = np.zeros((n_h * n_w, WS2, WS2), dtype=np.float32)
    masks[-n_w:] += ul
    masks[n_w - 1::n_w] += lr
    return masks


def _prep(x):
    # (8, 4096, 768) -> (8, 6, 128, 4096): [b, hp, (h2,d), (n,s)] dim-major windows
    img = np.asarray(x).transpose(0, 2, 1).reshape(B, 768, 64, 64)
    img = np.roll(img, (-DISP, -DISP), axis=(2, 3))
    t = img.reshape(B, HEADS, HD, 8, 8, 8, 8)          # b h d ny wy nx wx
    t = t.transpose(0, 1, 2, 3, 5, 4, 6)               # b h d ny nx wy wx
    t = t.reshape(B, HEADS, HD, NWIN, WS2)             # b h d n s
    t = t.reshape(B, HP, 2 * HD, NWIN * WS2)           # [b, hp, (h2,d), (n,s)]
    return np.ascontiguousarray(t.astype(np.float32))


def _build_nc():
    nc = bacc.Bacc(None)
    qdm = nc.dram_tensor("qdm", [HP, 128, 4096], mybir.dt.float32, kind="ExternalInput")
    kdm = nc.dram_tensor("kdm", [HP, 128, 4096], mybir.dt.float32, kind="ExternalInput")
    vdm = nc.dram_tensor("vdm", [HP, 128, 4096], mybir.dt.float32, kind="ExternalInput")
    bdq = nc.dram_tensor("bdq", [128, 128], mybir.dt.float32, kind="ExternalInput")
    bdk = nc.dram_tensor("bdk", [128, 128], mybir.dt.float32, kind="ExternalInput")
    bdv = nc.dram_tensor("bdv", [128, 128], mybir.dt.float32, kind="ExternalInput")
    wlt = nc.dram_tensor("wlt", [64, 64], mybir.dt.float32, kind="ExternalInput")
    bt = nc.dram_tensor("bt", [64, 4096], mybir.dt.float32, kind="ExternalInput")
    out_t = nc.dram_tensor("out_t", [768, 4096], mybir.dt.float32, kind="ExternalOutput")
    attn_e = nc.dram_tensor("attn_e", [HEADS, 64, 4096], mybir.dt.float32, kind="ExternalOutput")

    fp32 = mybir.dt.float32
    with tile.TileContext(nc) as tc:
        with tc.tile_pool(name="const", bufs=1) as cpool, \
             tc.tile_pool(name="qkv", bufs=1) as qkvp, \
             tc.tile_pool(name="proj", bufs=1) as projp, \
             tc.tile_pool(name="attn", bufs=2) as attnp, \
             tc.tile_pool(name="small", bufs=4) as smallp, \
             tc.tile_pool(name="ps", bufs=6, space="PSUM") as pp:

            bdq_sb = cpool.tile([128, 128], fp32)
            bdk_sb = cpool.tile([128, 128], fp32)
            bdv_sb = cpool.tile([128, 128], fp32)
            wlt_sb = cpool.tile([64, 64], fp32)
            bt_sb = cpool.tile([64, 4096], fp32)
            nc.sync.dma_start(out=bdq_sb[:], in_=bdq[:])
            nc.sync.dma_start(out=bdk_sb[:], in_=bdk[:])
            nc.sync.dma_start(out=bdv_sb[:], in_=bdv[:])
            nc.sync.dma_start(out=wlt_sb[:], in_=wlt[:])
            nc.sync.dma_start(out=bt_sb[:], in_=bt[:])

            for hp in range(HP):
                q_sb = qkvp.tile([128, 4096], fp32)
                k_sb = qkvp.tile([128, 4096], fp32)
                v_sb = qkvp.tile([128, 4096], fp32)
                nc.sync.dma_start(out=q_sb[:], in_=qdm[hp])
                nc.sync.dma_start(out=k_sb[:], in_=kdm[hp])
                nc.sync.dma_start(out=v_sb[:], in_=vdm[hp])

                qp_sb = projp.tile([128, 4096], fp32)
                kp_sb = projp.tile([128, 4096], fp32)
                vp_sb = projp.tile([64, 8192], fp32)

                # projections q,k: dim-major [(2h e), t]
                for c in range(8):
                    sl = slice(512 * c, 512 * (c + 1))
                    psq = pp.tile([128, 512], fp32, tag="ps")
                    nc.tensor.matmul(psq[:], bdq_sb[:], q_sb[:, sl], start=True, stop=True)
                    nc.scalar.copy(qp_sb[:, sl], psq[:])
                    psk = pp.tile([128, 512], fp32, tag="ps")
                    nc.tensor.matmul(psk[:], bdk_sb[:], k_sb[:, sl], start=True, stop=True)
                    nc.vector.tensor_copy(kp_sb[:, sl], psk[:])

                # v projection: token-major per window [64 tok, (2h e)]
                for g in range(16):  # 4 windows per PSUM bank
                    vps = pp.tile([64, 512], fp32, tag="ps")
                    for j in range(4):
                        n = 4 * g + j
                        nc.tensor.matmul(
                            vps[:, 128 * j:128 * (j + 1)],
                            v_sb[:, 64 * n:64 * (n + 1)], bdv_sb[:],
                            start=True, stop=True)
                    if g % 2 == 0:
                        nc.scalar.copy(vp_sb[:, 512 * g:512 * (g + 1)], vps[:])
                    else:
                        nc.vector.tensor_copy(vp_sb[:, 512 * g:512 * (g + 1)], vps[:])

                for h2 in range(2):
                    h = 2 * hp + h2
                    hsl = slice(64 * h2, 64 * (h2 + 1))
                    at_sb = attnp.tile([64, 4096], fp32)
                    # scores S^T[k,q] = exp(qk/8 + bias + mask), 8 windows per group
                    for g in range(8):
                        sps = pp.tile([64, 512], fp32, tag="ps")
                        for j in range(8):
                            n = 8 * g + j
                            wsl = slice(64 * n, 64 * (n + 1))
                            nc.tensor.matmul(
                                sps[:, 64 * j:64 * (j + 1)],
                                kp_sb[hsl, wsl], qp_sb[hsl, wsl],
                                start=True, stop=True)
                        gsl = slice(512 * g, 512 * (g + 1))
                        tmp = smallp.tile([64, 512], fp32)
                        nc.vector.tensor_add(tmp[:], sps[:], bt_sb[:, gsl])
                        nc.scalar.activation(at_sb[:, gsl], tmp[:],
                                             func=mybir.ActivationFunctionType.Exp)
                    nc.sync.dma_start(out=attn_e[h], in_=at_sb[:])

                    # AV: o^T[e, (n,q)] then Wlin per 8-window chunk
                    for g in range(8):
                        avps = pp.tile([64, 512], fp32, tag="ps")
                        for j in range(8):
                            n = 8 * g + j
                            nc.tensor.matmul(
                                avps[:, 64 * j:64 * (j + 1)],
                                vp_sb[:, 128 * n + 64 * h2:128 * n + 64 * (h2 + 1)],
                                at_sb[:, 64 * n:64 * (n + 1)],
                                start=True, stop=True)
                        o_sb = smallp.tile([64, 512], fp32)
                        if g % 2 == 0:
                            nc.vector.tensor_copy(o_sb[:], avps[:])
                        else:
                            nc.scalar.copy(o_sb[:], avps[:])
                        wps = pp.tile([64, 512], fp32, tag="ps")
                        nc.tensor.matmul(wps[:], wlt_sb[:], o_sb[:], start=True, stop=True)
                        ot_sb = smallp.tile([64, 512], fp32)
                        if g % 2 == 0:
                            nc.scalar.copy(ot_sb[:], wps[:])
                        else:
                            nc.vector.tensor_copy(ot_sb[:], wps[:])
                        nc.sync.dma_start(
                            out=out_t[64 * h:64 * (h + 1), 512 * g:512 * (g + 1)],
                            in_=ot_sb[:])
    nc.finalize()
    return nc


_NC_CACHE = None
LAST_EXEC_NS = None


def kernel(q_feat, k_feat, v_feat, Wq, Wk, Wv, Wlin, pe, h, w):
    global _NC_CACHE
    Wq, Wk, Wv, Wlin, pe = [np.asarray(a, np.float32) for a in (Wq, Wk, Wv, Wlin, pe)]
    qdm = _prep(q_feat)
    kdm = _prep(k_feat)
    vdm = _prep(v_feat)

    eye2 = np.eye(2, dtype=np.float32)
    bdq = np.kron(eye2, Wq.T * 0.125).astype(np.float32)
    bdk = np.kron(eye2, Wk.T).astype(np.float32)
    bdv = np.kron(eye2, Wv.T).astype(np.float32)
    wlt = np.ascontiguousarray(Wlin.T.astype(np.float32))

    rel = _rel_indices()
    bias = pe[rel[:, :, 0], rel[:, :, 1]].astype(np.float32)  # [q,k]
    masks = _window_masks(8, 8)
    masks = np.where(np.isneginf(masks), np.float32(-30000.0), masks).astype(np.float32)
    Bmat = bias[None, :, :] + masks                       # [n,q,k]
    bt = np.ascontiguousarray(Bmat.transpose(2, 0, 1).reshape(64, 4096))  # [k,(n,q)]

    if _NC_CACHE is None:
        _NC_CACHE = _build_nc()
    nc = _NC_CACHE

    core_ids = list(range(8))
    in_maps = []
    for b in range(B):
        in_maps.append({
            "qdm": np.ascontiguousarray(qdm[b]),
            "kdm": np.ascontiguousarray(kdm[b]),
            "vdm": np.ascontiguousarray(vdm[b]),
            "bdq": bdq, "bdk": bdk, "bdv": bdv, "wlt": wlt, "bt": bt,
        })
    global LAST_EXEC_NS
    import time as _time
    want_trace = os.environ.get("KBENCH_TRACE") == "1"
    t0 = _time.time()
    try:
        robj = run_bass_kernel_spmd(nc, in_maps, core_ids, trace=want_trace)
    except ModuleNotFoundError:
        robj = run_bass_kernel_spmd(nc, in_maps, core_ids)
    t1 = _time.time()
    LAST_EXEC_NS = robj.exec_time_ns if robj.exec_time_ns else int((t1 - t0) * 1e9)
    res = robj.results

    out = np.empty((B, 4096, 768), dtype=np.float32)
    attn = np.empty((B, HEADS, NWIN, WS2, WS2), dtype=np.float32)
    for b in range(B):
        E = np.asarray(res[b]["attn_e"])                   # [h, k, (n,q)]
        A = E.reshape(HEADS, 64, 64, 64).transpose(0, 2, 3, 1)  # [h,n,q,k]
        r = A.sum(axis=-1, keepdims=True)                  # [h,n,q,1]
        attn[b] = A / r
        ot = np.asarray(res[b]["out_t"]).reshape(HEADS, 64, 64, 64)  # [h,e,n,s]
        ot = ot / r[:, None, :, :, 0]                      # [h,e,n,s] / [h,1,n,s]
        t = ot.reshape(HEADS, 64, 8, 8, 8, 8)              # h e ny nx wy wx
        t = t.transpose(0, 1, 2, 4, 3, 5).reshape(768, 64, 64)
        t = np.roll(t, (DISP, DISP), axis=(1, 2))
        out[b] = t.reshape(768, 4096).T
    return out, attn
